# revision 17
# baseline (speedup 1.0000x reference)
"""Trainium2 Bass kernel for nn_BatchRelationalModule (gnn_message_passing).

Reference computation (per batch b of 32):
  x = [imgfeat(128) | coord] per position l in 0..143            # [L, 129]
  gi = x @ W1[:129]   (indexed by j);  gjb = x @ W1[129:] + b1   # [L, 64]
  A[:, (i,j)] = lrelu(gi[j] + gjb[i])                            # [64, L*L]
  P = W2.T @ A + b2;  s = sum_{i,j} lrelu(P)                     # [64]
  out = lrelu(lrelu(s @ Wp + bp) @ Wo + bo)                      # [64]

Sharding: data-parallel over batch, 4 batches per core (2 groups of 2
batches stacked on SBUF partitions: rows 0-63 = even batch features,
rows 64-127 = odd batch features).

Per-core device pipeline (per 2-batch group):
  PE  : gi/gjb prep matmuls (x @ W, fp16) into PSUM halves
  DVE : custom fused op  Z = lrelu(gi_bcast + gjb_bcast)  (one pass,
        broadcast via 0-stride access patterns, fp32 in / fp16 out)
        + accum_out = rowsum(Z) in fp32
  PE  : W2.T @ Z as bf16 hi+lo accumulating matmul pairs (~16-bit
        effective weight mantissa; bf16 keeps the fp32 exponent range so
        the lo part never denormal-flushes), col-tiled so both batches
        run concurrently and PSUM packs [128, fd]
  ACT : relu(0.99*(P + b2)) with per-partition bias + fused accum_out
  final: sum lrelu(P+b2) = 0.01*(W2.T @ rowsum(Z) + Npair*b2) + accum(relu),
  assembled with tiny per-batch matmuls (identity-matmul moves the
  odd-batch partition halves), then the small MLP on PE/DVE (fp32).

All constants arrive in 4 packed DMA transfers (per-transfer overhead
~0.6us dominates small loads).
"""

import os
import sys

import numpy as np

for _p in ("/opt/trn_rl_repo",):
    if os.path.isdir(_p) and _p not in sys.path:
        sys.path.insert(0, _p)

import operator

import concourse.bass as bass
import concourse.tile as tile
from concourse import bacc, mybir
from concourse.bass import _add_dep_helper

B, C = 32, 128
L = 144
HID = 64
NCORES = 8
BPC = 4  # batches per core
NPAIR = L * L  # 20736
SLOPE = 0.01
LIN_COEF = SLOPE          # weight of the exact linear term
RELU_COEF = 1.0 - SLOPE   # weight of the relu-sum term
PSUM_FD = 2048
SCH = [16, 32, 96]  # i-chunk sizes: ramp up so the consumer never stalls
N_PTILES = (NPAIR + PSUM_FD - 1) // PSUM_FD  # 11 psum tiles per group

# fp32 constant pack column map
_C_GA2 = 0          # [128, 144]
_C_GB2 = 144        # [128, 144]
_C_B2C = 288        # [128, 1]
_C_W2S = 289        # [128, 64] (0.01*W2 duplicated into both halves)
_C_I64 = 353        # [128, 64] (identity duplicated into both halves)
_C_WP = 417         # [64, 64]
_C_WO = 481         # [64, 64]
_C_BP4 = 545        # [64, 4]
_C_BO4 = 549        # [64, 4]
_C_C2 = 553         # [64, 1] (0.01 * NPAIR * b2 as a per-partition column)
_C32_COLS = 554

_cache: dict = {}


def _register_op():
    """Register the fused lrelu(Src0 + Src1) custom DVE op at runtime."""
    from concourse import dve_ops
    from concourse.dve_spec import Spec, Src0, Src1, C0, maxx, lower, _has_src1
    from concourse.dve_uop import DveOpSpec

    name = "LRELU_ADD_ANT"
    if name in dve_ops._SUB_OPCODE_FOR_NAME:
        return next(o for o in dve_ops.OPS if o.name == name)

    def _ref(in0, in1, s0, s1, imm2):
        z = np.asarray(in0, np.float32) + np.asarray(in1, np.float32)
        s0v = s0 if isinstance(s0, float) else np.asarray(s0, np.float32)
        out = np.maximum(z, z * s0v)
        acc = out.reshape(out.shape[0], -1).sum(axis=-1, keepdims=True)
        return out, acc.astype(np.float32)

    _z = Src0 + Src1
    spec = Spec(body=maxx(_z, _z * C0), accum=operator.add, reference=_ref)
    op = dve_ops.DveOp(name, spec, subdim=False, uops_sha={})
    dve_ops.OPS.append(op)
    row = dve_ops._CUSTOM_DVE_ROW_BASE + len(dve_ops.OPS) - 1
    assert row < 0x20
    dve_ops._SUB_OPCODE_FOR_NAME[name] = row
    dve_ops.CUSTOM_DVE_SPECS[name] = spec
    for ver in ("v3", "v4"):
        try:
            uops = lower(spec, ver=ver)
            sha = DveOpSpec(
                name=name, opcode=row, uops=uops, rd1_en=_has_src1(spec)
            ).sha(ver)
            op.uops_sha[ver] = sha
        except Exception:
            pass
    return op


def _bcast_in0(ap, S):
    """[128, L] -> [128, S, L] repeating the whole tile S times (0-stride)."""
    return bass.AP(ap.tensor, ap.offset, [ap.ap[0], [0, S], *ap.ap[1:]])


def _bcast_in1(ap, n_inner):
    """[128, S] -> [128, S, n_inner] repeating each column (0-stride inner)."""
    return bass.AP(ap.tensor, ap.offset, [*ap.ap, [0, n_inner]])


def build_nc():
    """Build the Bass module (identical for every core)."""
    LRELU = _register_op()
    nc = bacc.Bacc(trn_type="TRN2")
    f32 = mybir.dt.float32
    f16 = mybir.dt.float16
    bf16 = mybir.dt.bfloat16
    AF = mybir.ActivationFunctionType

    d_xall = nc.dram_tensor("xall", [BPC, 128, L], f16, kind="ExternalInput")
    d_pk16 = nc.dram_tensor("pk16", [128, 2 * HID], f16, kind="ExternalInput")
    d_pkbf = nc.dram_tensor("pkbf", [128, 2 * HID], bf16, kind="ExternalInput")
    d_pk32 = nc.dram_tensor("pk32", [128, _C32_COLS], f32, kind="ExternalInput")
    d_out = nc.dram_tensor("out", [HID, BPC], f32, kind="ExternalOutput")

    with tile.TileContext(nc) as tc:
        with (
            tc.tile_pool(name="const", bufs=1) as cp,
            tc.tile_pool(name="g", bufs=2) as gp,
            tc.tile_pool(name="zl", bufs=3) as zlp,
            tc.tile_pool(name="trash", bufs=2) as trp,
            tc.tile_pool(name="small", bufs=1) as smp,
            tc.tile_pool(name="psum", bufs=2, space=bass.MemorySpace.PSUM) as pp,
        ):
            xall = cp.tile([128, BPC * L], f16, tag="xall")
            # pack 4 batches along the free dim in one transfer
            src = d_xall[:]  # [BPC, 128, L]
            src_perm = bass.AP(
                src.tensor, src.offset, [src.ap[1], src.ap[0], src.ap[2]]
            )
            nc.sync.dma_start(xall[:], src_perm)
            pk16 = cp.tile([128, 2 * HID], f16, tag="pk16")
            nc.sync.dma_start(pk16[:], d_pk16[:])
            pkbf = cp.tile([128, 2 * HID], bf16, tag="pkbf")
            nc.sync.dma_start(pkbf[:], d_pkbf[:])
            pk32 = cp.tile([128, _C32_COLS], f32, tag="pk32")
            nc.sync.dma_start(pk32[:], d_pk32[:])

            t_xf = [xall[:, L * b : L * (b + 1)] for b in range(BPC)]
            t_wa = pk16[:, 0:HID]
            t_wb = pk16[:, HID : 2 * HID]
            t_whi = pkbf[:, 0:HID]
            t_wlo = pkbf[:, HID : 2 * HID]
            t_ga2 = pk32[:, _C_GA2 : _C_GA2 + L]
            t_gb2 = pk32[:, _C_GB2 : _C_GB2 + L]
            t_b2c = pk32[:, _C_B2C : _C_B2C + 1]
            t_w2s = pk32[:, _C_W2S : _C_W2S + HID]
            t_i64 = pk32[:, _C_I64 : _C_I64 + HID]
            t_wp = pk32[0:HID, _C_WP : _C_WP + HID]
            t_wo = pk32[0:HID, _C_WO : _C_WO + HID]
            t_bp4 = pk32[0:HID, _C_BP4 : _C_BP4 + BPC]
            t_bo4 = pk32[0:HID, _C_BO4 : _C_BO4 + BPC]
            t_c2 = pk32[0:HID, _C_C2 : _C_C2 + 1]

            accz = smp.tile([128, 8], f32, tag="accz")
            absc = smp.tile([128, 32], f32, tag="absc")
            zsumg = smp.tile([128, 2], f32, tag="zsumg")
            asumg = smp.tile([128, 2], f32, tag="asumg")

            # ---- prep: gi2 / gjb2 for both groups (PSUM halves per batch) --
            gi2s, gjb2s = [], []
            for g in range(2):
                ps_gi = pp.tile([128, L], f32, tag="mm")
                nc.tensor.matmul(ps_gi[0:64, :], t_wa, t_xf[2 * g])
                nc.tensor.matmul(ps_gi[64:128, :], t_wa, t_xf[2 * g + 1])
                gi2 = gp.tile([128, L], f32, tag="gi2")
                nc.vector.tensor_add(gi2[:], ps_gi[:], t_ga2)
                ps_gj = pp.tile([128, L], f32, tag="mm")
                nc.tensor.matmul(ps_gj[0:64, :], t_wb, t_xf[2 * g])
                nc.tensor.matmul(ps_gj[64:128, :], t_wb, t_xf[2 * g + 1])
                gjb2 = gp.tile([128, L], f32, tag="gjb2")
                nc.vector.tensor_add(gjb2[:], ps_gj[:], t_gb2)
                gi2s.append(gi2)
                gjb2s.append(gjb2)

            # ---- main: per group, fused-lrelu Z tiles -> matmuls -> ACT ----
            for g in range(2):
                gi2, gjb2 = gi2s[g], gjb2s[g]
                segs = []  # (tile, start_col, n_cols)
                i0 = 0
                zi_insts = []
                for ci, S in enumerate(SCH):
                    zt = zlp.tile([128, S * L], bf16, tag="zl")
                    in0 = _bcast_in0(gi2[:, 0:L], S)
                    in1 = _bcast_in1(gjb2[:, i0 : i0 + S], L)
                    zi = nc.vector._custom_dve(
                        LRELU,
                        out=zt[:],
                        in0=in0,
                        in1=in1,
                        s0=SLOPE,
                        accum_out=accz[:, 4 * g + ci : 4 * g + ci + 1],
                    )
                    zi_insts.append(zi)
                    segs.append((zt, i0 * L, S * L))
                    i0 += S

                def seg_for(c):
                    for (zt, s0_, n_) in segs:
                        if s0_ <= c < s0_ + n_:
                            return zt, c - s0_, s0_ + n_ - c
                    raise AssertionError(c)

                c = 0
                ti = 0
                ps = None
                act_insts = []
                while c < NPAIR:
                    pcol = c % PSUM_FD
                    if pcol == 0:
                        ps = pp.tile([128, PSUM_FD], f32, tag="mm")
                    zt, zoff, zleft = seg_for(c)
                    n = min(512 - (pcol % 512), zleft, NPAIR - c)
                    for h in range(2):  # batch half: partitions 64h..64h+63
                        r = slice(64 * h, 64 * h + 64)
                        nc.tensor.matmul(
                            ps[r, pcol : pcol + n],
                            t_whi[r, :],
                            zt[r, zoff : zoff + n],
                            start=True,
                            stop=False,
                        )
                        nc.tensor.matmul(
                            ps[r, pcol : pcol + n],
                            t_wlo[r, :],
                            zt[r, zoff : zoff + n],
                            start=False,
                            stop=True,
                        )
                    c += n
                    if c % PSUM_FD == 0 or c == NPAIR:
                        fd = pcol + n
                        tr = trp.tile([128, PSUM_FD], f16, tag="tr")
                        ai = nc.scalar.activation(
                            tr[:, 0:fd],
                            ps[:, 0:fd],
                            AF.Relu,
                            bias=t_b2c,
                            scale=RELU_COEF,
                            accum_out=absc[:, 16 * g + ti : 16 * g + ti + 1],
                        )
                        act_insts.append(ai)
                        ti += 1
                assert ti == N_PTILES

                # per-group: fold the per-chunk accumulators. The reduces
                # must wait for the accum_out (second-output) writes, which
                # Tile's dependency tracker does not see — add explicit edges.
                rz = nc.vector.tensor_reduce(
                    zsumg[:, g : g + 1],
                    accz[:, 4 * g : 4 * g + len(SCH)],
                    axis=mybir.AxisListType.X,
                    op=mybir.AluOpType.add,
                )
                for zi in zi_insts:
                    _add_dep_helper(rz.ins, zi.ins, sync=True, reason="accz accum_out")
                ra = nc.vector.tensor_reduce(
                    asumg[:, g : g + 1],
                    absc[:, 16 * g : 16 * g + N_PTILES],
                    axis=mybir.AxisListType.X,
                    op=mybir.AluOpType.add,
                )
                for ai in act_insts:
                    _add_dep_helper(ra.ins, ai.ins, sync=True, reason="absc accum_out")

            # ---- tail: s = 0.01*(W2.T zsum + N b2) + relu-accum, tiny MLP --
            zsum_all = smp.tile([HID, BPC], f32, tag="zsum_all")
            asum_all = smp.tile([HID, BPC], f32, tag="asum_all")
            for b in range(BPC):
                g, h = divmod(b, 2)
                r = slice(64 * h, 64 * h + 64)
                nc.sync.dma_start(zsum_all[0:64, b : b + 1], zsumg[r, g : g + 1])
                nc.sync.dma_start(asum_all[0:64, b : b + 1], asumg[r, g : g + 1])
            lz = pp.tile([HID, BPC], f32, tag="mm")
            nc.tensor.matmul(lz[:], t_w2s[0:HID, :], zsum_all[:])
            s_all = smp.tile([HID, BPC], f32, tag="s_all")
            nc.vector.tensor_scalar_add(s_all[:], lz[:], t_c2)
            nc.vector.tensor_add(s_all[:], s_all[:], asum_all[:])

            p1 = pp.tile([HID, BPC], f32, tag="mm")
            nc.tensor.matmul(p1[:], t_wp, s_all[:])
            h1 = smp.tile([HID, BPC], f32, tag="h1")
            nc.vector._custom_dve(LRELU, out=h1[:], in0=p1[:], in1=t_bp4, s0=SLOPE)
            p2 = pp.tile([HID, BPC], f32, tag="mm")
            nc.tensor.matmul(p2[:], t_wo, h1[:])
            fin = smp.tile([HID, BPC], f32, tag="fin")
            nc.vector._custom_dve(LRELU, out=fin[:], in0=p2[:], in1=t_bo4, s0=SLOPE)
            nc.sync.dma_start(d_out[:], fin[:])

    nc.compile()
    return nc


def host_prep(inputs):
    """Host-side weight preprocessing -> shared input map + per-core xall."""
    x_img = np.asarray(inputs["x_img"], np.float32)
    W1 = np.asarray(inputs["W1"], np.float32)
    b1 = np.asarray(inputs["b1"], np.float32)
    W2 = np.asarray(inputs["W2"], np.float32)
    b2 = np.asarray(inputs["b2"], np.float32)
    Wp = np.asarray(inputs["Wp"], np.float32)
    bp = np.asarray(inputs["bp"], np.float32)
    Wo = np.asarray(inputs["Wo"], np.float32)
    bo = np.asarray(inputs["bo"], np.float32)
    import ml_dtypes

    BF = ml_dtypes.bfloat16

    x = x_img.reshape(B, C, L)  # [b, c, l]
    coords = np.arange(L, dtype=np.float32)
    GaT = (coords[:, None] * W1[C][None, :]).T  # [64, 144]
    GbT = (coords[:, None] * W1[C + 1 + C][None, :] + b1[None, :]).T
    W2hi = W2.astype(BF)
    W2lo = (W2 - W2hi.astype(np.float32)).astype(BF)

    pk16 = np.zeros((128, 2 * HID), np.float16)
    pk16[:, 0:HID] = W1[:C].astype(np.float16)
    pk16[:, HID : 2 * HID] = W1[C + 1 : C + 1 + C].astype(np.float16)

    pkbf = np.zeros((128, 2 * HID), BF)
    pkbf[0:64, 0:HID] = W2hi
    pkbf[64:128, 0:HID] = W2hi
    pkbf[0:64, HID:] = W2lo
    pkbf[64:128, HID:] = W2lo

    pk32 = np.zeros((128, _C32_COLS), np.float32)
    pk32[:, _C_GA2 : _C_GA2 + L] = np.concatenate([GaT, GaT], 0)
    pk32[:, _C_GB2 : _C_GB2 + L] = np.concatenate([GbT, GbT], 0)
    pk32[:, _C_B2C] = np.tile(RELU_COEF * b2, 2)
    pk32[0:64, _C_W2S : _C_W2S + HID] = LIN_COEF * W2
    pk32[64:128, _C_W2S : _C_W2S + HID] = LIN_COEF * W2
    eye = np.eye(HID, dtype=np.float32)
    pk32[0:64, _C_I64 : _C_I64 + HID] = eye
    pk32[64:128, _C_I64 : _C_I64 + HID] = eye
    pk32[0:HID, _C_WP : _C_WP + HID] = Wp
    pk32[0:HID, _C_WO : _C_WO + HID] = Wo
    pk32[0:HID, _C_BP4 : _C_BP4 + BPC] = np.repeat(bp[:, None], BPC, axis=1)
    pk32[0:HID, _C_BO4 : _C_BO4 + BPC] = np.repeat(bo[:, None], BPC, axis=1)
    pk32[0:HID, _C_C2] = LIN_COEF * NPAIR * b2

    base = {
        "pk16": np.ascontiguousarray(pk16),
        "pkbf": np.ascontiguousarray(pkbf),
        "pk32": np.ascontiguousarray(pk32),
    }
    in_maps = []
    for k in range(NCORES):
        m = dict(base)
        m["xall"] = np.ascontiguousarray(
            x[BPC * k : BPC * (k + 1)].astype(np.float16)
        )
        in_maps.append(m)
    return in_maps


def kernel(**inputs) -> np.ndarray:
    from concourse.bass_utils import run_bass_kernel_spmd

    if "nc" not in _cache:
        _cache["nc"] = build_nc()
    nc = _cache["nc"]
    in_maps = host_prep(inputs)
    res = run_bass_kernel_spmd(nc, in_maps, core_ids=list(range(NCORES)))
    out = np.concatenate([r["out"].T for r in res.results], axis=0)  # [32, 64]
    return np.ascontiguousarray(out, np.float32)


# revision 18
# speedup vs baseline: 1.0459x; 1.0459x over previous
"""Trainium2 Bass kernel for nn_BatchRelationalModule (gnn_message_passing).

Reference computation (per batch b of 32):
  x = [imgfeat(128) | coord] per position l in 0..143            # [L, 129]
  gi = x @ W1[:129]   (indexed by j);  gjb = x @ W1[129:] + b1   # [L, 64]
  A[:, (i,j)] = lrelu(gi[j] + gjb[i])                            # [64, L*L]
  P = W2.T @ A + b2;  s = sum_{i,j} lrelu(P)                     # [64]
  out = lrelu(lrelu(s @ Wp + bp) @ Wo + bo)                      # [64]

Sharding: data-parallel over batch, 4 batches per core (2 groups of 2
batches stacked on SBUF partitions: rows 0-63 = even batch features,
rows 64-127 = odd batch features).

Per-core device pipeline (per 2-batch group):
  PE  : gi/gjb prep matmuls (x @ W, fp16) into PSUM halves
  DVE : custom fused op  Z = lrelu(gi_bcast + gjb_bcast)  (one pass,
        broadcast via 0-stride access patterns, fp32 in / fp16 out)
        + accum_out = rowsum(Z) in fp32
  PE  : W2.T @ Z as bf16 hi+lo accumulating matmul pairs (~16-bit
        effective weight mantissa; bf16 keeps the fp32 exponent range so
        the lo part never denormal-flushes), col-tiled so both batches
        run concurrently and PSUM packs [128, fd]
  ACT : relu(0.99*(P + b2)) with per-partition bias + fused accum_out
  final: sum lrelu(P+b2) = 0.01*(W2.T @ rowsum(Z) + Npair*b2) + accum(relu),
  assembled with tiny per-batch matmuls (identity-matmul moves the
  odd-batch partition halves), then the small MLP on PE/DVE (fp32).

All constants arrive in 4 packed DMA transfers (per-transfer overhead
~0.6us dominates small loads).
"""

import os
import sys

import numpy as np

for _p in ("/opt/trn_rl_repo",):
    if os.path.isdir(_p) and _p not in sys.path:
        sys.path.insert(0, _p)

import operator

import concourse.bass as bass
import concourse.tile as tile
from concourse import bacc, mybir
from concourse.bass import _add_dep_helper

B, C = 32, 128
L = 144
HID = 64
NCORES = 8
BPC = 4  # batches per core
NPAIR = L * L  # 20736
SLOPE = 0.01
LIN_COEF = SLOPE          # weight of the exact linear term
RELU_COEF = 1.0 - SLOPE   # weight of the relu-sum term
PSUM_FD = 2048
SCH = [8, 16, 24, 32, 64]  # i-chunk ramp: producer stays ahead of consumers
N_PTILES = (NPAIR + PSUM_FD - 1) // PSUM_FD  # 11 psum tiles per group

# fp32 constant pack column map
_C_GA2 = 0          # [128, 144]
_C_GB2 = 144        # [128, 144]
_C_B2C = 288        # [128, 1]
_C_W2S = 289        # [128, 64] (0.01*W2 duplicated into both halves)
_C_I64 = 353        # [128, 64] (identity duplicated into both halves)
_C_WP = 417         # [64, 64]
_C_WO = 481         # [64, 64]
_C_BP4 = 545        # [64, 4]
_C_BO4 = 549        # [64, 4]
_C_C2 = 553         # [64, 1] (0.01 * NPAIR * b2 as a per-partition column)
_C32_COLS = 554

_cache: dict = {}


def _register_op():
    """Register the fused lrelu(Src0 + Src1) custom DVE op at runtime."""
    from concourse import dve_ops
    from concourse.dve_spec import Spec, Src0, Src1, C0, maxx, lower, _has_src1
    from concourse.dve_uop import DveOpSpec

    name = "LRELU_ADD_ANT"
    if name in dve_ops._SUB_OPCODE_FOR_NAME:
        return next(o for o in dve_ops.OPS if o.name == name)

    def _ref(in0, in1, s0, s1, imm2):
        z = np.asarray(in0, np.float32) + np.asarray(in1, np.float32)
        s0v = s0 if isinstance(s0, float) else np.asarray(s0, np.float32)
        out = np.maximum(z, z * s0v)
        acc = out.reshape(out.shape[0], -1).sum(axis=-1, keepdims=True)
        return out, acc.astype(np.float32)

    _z = Src0 + Src1
    spec = Spec(body=maxx(_z, _z * C0), accum=operator.add, reference=_ref)
    op = dve_ops.DveOp(name, spec, subdim=False, uops_sha={})
    dve_ops.OPS.append(op)
    row = dve_ops._CUSTOM_DVE_ROW_BASE + len(dve_ops.OPS) - 1
    assert row < 0x20
    dve_ops._SUB_OPCODE_FOR_NAME[name] = row
    dve_ops.CUSTOM_DVE_SPECS[name] = spec
    for ver in ("v3", "v4"):
        try:
            uops = lower(spec, ver=ver)
            sha = DveOpSpec(
                name=name, opcode=row, uops=uops, rd1_en=_has_src1(spec)
            ).sha(ver)
            op.uops_sha[ver] = sha
        except Exception:
            pass
    return op


def _bcast_in0(ap, S):
    """[128, L] -> [128, S, L] repeating the whole tile S times (0-stride)."""
    return bass.AP(ap.tensor, ap.offset, [ap.ap[0], [0, S], *ap.ap[1:]])


def _bcast_in1(ap, n_inner):
    """[128, S] -> [128, S, n_inner] repeating each column (0-stride inner)."""
    return bass.AP(ap.tensor, ap.offset, [*ap.ap, [0, n_inner]])


def build_nc():
    """Build the Bass module (identical for every core)."""
    LRELU = _register_op()
    nc = bacc.Bacc(trn_type="TRN2")
    f32 = mybir.dt.float32
    f16 = mybir.dt.float16
    bf16 = mybir.dt.bfloat16
    AF = mybir.ActivationFunctionType

    d_xall = nc.dram_tensor("xall", [BPC, 128, L], f16, kind="ExternalInput")
    d_pk16 = nc.dram_tensor("pk16", [128, 2 * HID], f16, kind="ExternalInput")
    d_pkbf = nc.dram_tensor("pkbf", [128, 2 * HID], bf16, kind="ExternalInput")
    d_pk32 = nc.dram_tensor("pk32", [128, _C32_COLS], f32, kind="ExternalInput")
    d_out = nc.dram_tensor("out", [HID, BPC], f32, kind="ExternalOutput")

    with tile.TileContext(nc) as tc:
        with (
            tc.tile_pool(name="const", bufs=1) as cp,
            tc.tile_pool(name="g", bufs=2) as gp,
            tc.tile_pool(name="zl", bufs=3) as zlp,
            tc.tile_pool(name="trash", bufs=2) as trp,
            tc.tile_pool(name="small", bufs=1) as smp,
            tc.tile_pool(name="psum", bufs=2, space=bass.MemorySpace.PSUM) as pp,
        ):
            xall = cp.tile([128, BPC * L], f16, tag="xall")
            # pack 4 batches along the free dim in one transfer
            src = d_xall[:]  # [BPC, 128, L]
            src_perm = bass.AP(
                src.tensor, src.offset, [src.ap[1], src.ap[0], src.ap[2]]
            )
            nc.sync.dma_start(xall[:], src_perm)
            pk16 = cp.tile([128, 2 * HID], f16, tag="pk16")
            nc.sync.dma_start(pk16[:], d_pk16[:])
            pkbf = cp.tile([128, 2 * HID], bf16, tag="pkbf")
            nc.sync.dma_start(pkbf[:], d_pkbf[:])
            pk32 = cp.tile([128, _C32_COLS], f32, tag="pk32")
            nc.sync.dma_start(pk32[:], d_pk32[:])

            t_xf = [xall[:, L * b : L * (b + 1)] for b in range(BPC)]
            t_wa = pk16[:, 0:HID]
            t_wb = pk16[:, HID : 2 * HID]
            t_whi = pkbf[:, 0:HID]
            t_wlo = pkbf[:, HID : 2 * HID]
            t_ga2 = pk32[:, _C_GA2 : _C_GA2 + L]
            t_gb2 = pk32[:, _C_GB2 : _C_GB2 + L]
            t_b2c = pk32[:, _C_B2C : _C_B2C + 1]
            t_w2s = pk32[:, _C_W2S : _C_W2S + HID]
            t_i64 = pk32[:, _C_I64 : _C_I64 + HID]
            t_wp = pk32[0:HID, _C_WP : _C_WP + HID]
            t_wo = pk32[0:HID, _C_WO : _C_WO + HID]
            t_bp4 = pk32[0:HID, _C_BP4 : _C_BP4 + BPC]
            t_bo4 = pk32[0:HID, _C_BO4 : _C_BO4 + BPC]
            t_c2 = pk32[0:HID, _C_C2 : _C_C2 + 1]

            accz = smp.tile([128, 16], f32, tag="accz")
            absc = smp.tile([128, 32], f32, tag="absc")
            zsumg = smp.tile([128, 2], f32, tag="zsumg")
            asumg = smp.tile([128, 2], f32, tag="asumg")

            # ---- prep: gi2 / gjb2 for both groups (PSUM halves per batch) --
            gi2s, gjb2s = [], []
            for g in range(2):
                ps_gi = pp.tile([128, L], f32, tag="mm")
                nc.tensor.matmul(ps_gi[0:64, :], t_wa, t_xf[2 * g])
                nc.tensor.matmul(ps_gi[64:128, :], t_wa, t_xf[2 * g + 1])
                gi2 = gp.tile([128, L], f32, tag="gi2")
                nc.vector.tensor_add(gi2[:], ps_gi[:], t_ga2)
                ps_gj = pp.tile([128, L], f32, tag="mm")
                nc.tensor.matmul(ps_gj[0:64, :], t_wb, t_xf[2 * g])
                nc.tensor.matmul(ps_gj[64:128, :], t_wb, t_xf[2 * g + 1])
                gjb2 = gp.tile([128, L], f32, tag="gjb2")
                nc.vector.tensor_add(gjb2[:], ps_gj[:], t_gb2)
                gi2s.append(gi2)
                gjb2s.append(gjb2)

            # ---- main: per group, fused-lrelu Z tiles -> matmuls -> ACT ----
            for g in range(2):
                gi2, gjb2 = gi2s[g], gjb2s[g]
                segs = []  # (tile, start_col, n_cols)
                i0 = 0
                zi_insts = []
                for ci, S in enumerate(SCH):
                    zt = zlp.tile([128, S * L], bf16, tag="zl")
                    in0 = _bcast_in0(gi2[:, 0:L], S)
                    in1 = _bcast_in1(gjb2[:, i0 : i0 + S], L)
                    zi = nc.vector._custom_dve(
                        LRELU,
                        out=zt[:],
                        in0=in0,
                        in1=in1,
                        s0=SLOPE,
                        accum_out=accz[:, 8 * g + ci : 8 * g + ci + 1],
                    )
                    zi_insts.append(zi)
                    segs.append((zt, i0 * L, S * L))
                    i0 += S

                def seg_for(c):
                    for (zt, s0_, n_) in segs:
                        if s0_ <= c < s0_ + n_:
                            return zt, c - s0_, s0_ + n_ - c
                    raise AssertionError(c)

                c = 0
                ti = 0
                ps = None
                act_insts = []
                while c < NPAIR:
                    pcol = c % PSUM_FD
                    if pcol == 0:
                        ps = pp.tile([128, PSUM_FD], f32, tag="mm")
                    zt, zoff, zleft = seg_for(c)
                    n = min(512 - (pcol % 512), zleft, NPAIR - c)
                    for h in range(2):  # batch half: partitions 64h..64h+63
                        r = slice(64 * h, 64 * h + 64)
                        nc.tensor.matmul(
                            ps[r, pcol : pcol + n],
                            t_whi[r, :],
                            zt[r, zoff : zoff + n],
                            start=True,
                            stop=False,
                        )
                        nc.tensor.matmul(
                            ps[r, pcol : pcol + n],
                            t_wlo[r, :],
                            zt[r, zoff : zoff + n],
                            start=False,
                            stop=True,
                        )
                    c += n
                    if c % PSUM_FD == 0 or c == NPAIR:
                        fd = pcol + n
                        tr = trp.tile([128, PSUM_FD], f16, tag="tr")
                        ai = nc.scalar.activation(
                            tr[:, 0:fd],
                            ps[:, 0:fd],
                            AF.Relu,
                            bias=t_b2c,
                            scale=RELU_COEF,
                            accum_out=absc[:, 16 * g + ti : 16 * g + ti + 1],
                        )
                        act_insts.append(ai)
                        ti += 1
                assert ti == N_PTILES

                # per-group: fold the per-chunk accumulators. The reduces
                # must wait for the accum_out (second-output) writes, which
                # Tile's dependency tracker does not see — add explicit edges.
                rz = nc.vector.tensor_reduce(
                    zsumg[:, g : g + 1],
                    accz[:, 8 * g : 8 * g + len(SCH)],
                    axis=mybir.AxisListType.X,
                    op=mybir.AluOpType.add,
                )
                for zi in zi_insts:
                    _add_dep_helper(rz.ins, zi.ins, sync=True, reason="accz accum_out")
                ra = nc.vector.tensor_reduce(
                    asumg[:, g : g + 1],
                    absc[:, 16 * g : 16 * g + N_PTILES],
                    axis=mybir.AxisListType.X,
                    op=mybir.AluOpType.add,
                )
                for ai in act_insts:
                    _add_dep_helper(ra.ins, ai.ins, sync=True, reason="absc accum_out")

            # ---- tail: s = 0.01*(W2.T zsum + N b2) + relu-accum, tiny MLP --
            zsum_all = smp.tile([HID, BPC], f32, tag="zsum_all")
            asum_all = smp.tile([HID, BPC], f32, tag="asum_all")
            for b in range(BPC):
                g, h = divmod(b, 2)
                r = slice(64 * h, 64 * h + 64)
                nc.sync.dma_start(zsum_all[0:64, b : b + 1], zsumg[r, g : g + 1])
                nc.sync.dma_start(asum_all[0:64, b : b + 1], asumg[r, g : g + 1])
            lz = pp.tile([HID, BPC], f32, tag="mm")
            nc.tensor.matmul(lz[:], t_w2s[0:HID, :], zsum_all[:])
            s_all = smp.tile([HID, BPC], f32, tag="s_all")
            nc.vector.tensor_scalar_add(s_all[:], lz[:], t_c2)
            nc.vector.tensor_add(s_all[:], s_all[:], asum_all[:])

            p1 = pp.tile([HID, BPC], f32, tag="mm")
            nc.tensor.matmul(p1[:], t_wp, s_all[:])
            h1 = smp.tile([HID, BPC], f32, tag="h1")
            nc.vector._custom_dve(LRELU, out=h1[:], in0=p1[:], in1=t_bp4, s0=SLOPE)
            p2 = pp.tile([HID, BPC], f32, tag="mm")
            nc.tensor.matmul(p2[:], t_wo, h1[:])
            fin = smp.tile([HID, BPC], f32, tag="fin")
            nc.vector._custom_dve(LRELU, out=fin[:], in0=p2[:], in1=t_bo4, s0=SLOPE)
            nc.sync.dma_start(d_out[:], fin[:])

    nc.compile()
    return nc


def host_prep(inputs):
    """Host-side weight preprocessing -> shared input map + per-core xall."""
    x_img = np.asarray(inputs["x_img"], np.float32)
    W1 = np.asarray(inputs["W1"], np.float32)
    b1 = np.asarray(inputs["b1"], np.float32)
    W2 = np.asarray(inputs["W2"], np.float32)
    b2 = np.asarray(inputs["b2"], np.float32)
    Wp = np.asarray(inputs["Wp"], np.float32)
    bp = np.asarray(inputs["bp"], np.float32)
    Wo = np.asarray(inputs["Wo"], np.float32)
    bo = np.asarray(inputs["bo"], np.float32)
    import ml_dtypes

    BF = ml_dtypes.bfloat16

    x = x_img.reshape(B, C, L)  # [b, c, l]
    coords = np.arange(L, dtype=np.float32)
    GaT = (coords[:, None] * W1[C][None, :]).T  # [64, 144]
    GbT = (coords[:, None] * W1[C + 1 + C][None, :] + b1[None, :]).T
    W2hi = W2.astype(BF)
    W2lo = (W2 - W2hi.astype(np.float32)).astype(BF)

    pk16 = np.zeros((128, 2 * HID), np.float16)
    pk16[:, 0:HID] = W1[:C].astype(np.float16)
    pk16[:, HID : 2 * HID] = W1[C + 1 : C + 1 + C].astype(np.float16)

    pkbf = np.zeros((128, 2 * HID), BF)
    pkbf[0:64, 0:HID] = W2hi
    pkbf[64:128, 0:HID] = W2hi
    pkbf[0:64, HID:] = W2lo
    pkbf[64:128, HID:] = W2lo

    pk32 = np.zeros((128, _C32_COLS), np.float32)
    pk32[:, _C_GA2 : _C_GA2 + L] = np.concatenate([GaT, GaT], 0)
    pk32[:, _C_GB2 : _C_GB2 + L] = np.concatenate([GbT, GbT], 0)
    pk32[:, _C_B2C] = np.tile(RELU_COEF * b2, 2)
    pk32[0:64, _C_W2S : _C_W2S + HID] = LIN_COEF * W2
    pk32[64:128, _C_W2S : _C_W2S + HID] = LIN_COEF * W2
    eye = np.eye(HID, dtype=np.float32)
    pk32[0:64, _C_I64 : _C_I64 + HID] = eye
    pk32[64:128, _C_I64 : _C_I64 + HID] = eye
    pk32[0:HID, _C_WP : _C_WP + HID] = Wp
    pk32[0:HID, _C_WO : _C_WO + HID] = Wo
    pk32[0:HID, _C_BP4 : _C_BP4 + BPC] = np.repeat(bp[:, None], BPC, axis=1)
    pk32[0:HID, _C_BO4 : _C_BO4 + BPC] = np.repeat(bo[:, None], BPC, axis=1)
    pk32[0:HID, _C_C2] = LIN_COEF * NPAIR * b2

    base = {
        "pk16": np.ascontiguousarray(pk16),
        "pkbf": np.ascontiguousarray(pkbf),
        "pk32": np.ascontiguousarray(pk32),
    }
    in_maps = []
    for k in range(NCORES):
        m = dict(base)
        m["xall"] = np.ascontiguousarray(
            x[BPC * k : BPC * (k + 1)].astype(np.float16)
        )
        in_maps.append(m)
    return in_maps


def kernel(**inputs) -> np.ndarray:
    from concourse.bass_utils import run_bass_kernel_spmd

    if "nc" not in _cache:
        _cache["nc"] = build_nc()
    nc = _cache["nc"]
    in_maps = host_prep(inputs)
    res = run_bass_kernel_spmd(nc, in_maps, core_ids=list(range(NCORES)))
    out = np.concatenate([r["out"].T for r in res.results], axis=0)  # [32, 64]
    return np.ascontiguousarray(out, np.float32)


# revision 19
# speedup vs baseline: 1.2126x; 1.1594x over previous
"""Trainium2 Bass kernel for nn_BatchRelationalModule (gnn_message_passing).

Reference computation (per batch b of 32):
  x = [imgfeat(128) | coord] per position l in 0..143            # [L, 129]
  gi = x @ W1[:129]   (indexed by j);  gjb = x @ W1[129:] + b1   # [L, 64]
  A[:, (i,j)] = lrelu(gi[j] + gjb[i])                            # [64, L*L]
  P = W2.T @ A + b2;  s = sum_{i,j} lrelu(P)                     # [64]
  out = lrelu(lrelu(s @ Wp + bp) @ Wo + bo)                      # [64]

Sharding: data-parallel over batch, 4 batches per core (2 groups of 2
batches stacked on SBUF partitions: rows 0-63 = even batch features,
rows 64-127 = odd batch features).

Per-core device pipeline (per 2-batch group):
  PE  : gi/gjb prep matmuls (x @ W, fp16) into PSUM halves
  DVE : custom fused op  Z = lrelu(gi_bcast + gjb_bcast)  (one pass,
        broadcast via 0-stride access patterns, fp32 in / fp16 out)
        + accum_out = rowsum(Z) in fp32
  PE  : W2.T @ Z as bf16 hi+lo accumulating matmul pairs (~16-bit
        effective weight mantissa; bf16 keeps the fp32 exponent range so
        the lo part never denormal-flushes), col-tiled so both batches
        run concurrently and PSUM packs [128, fd]
  ACT : relu(0.99*(P + b2)) with per-partition bias + fused accum_out
  final: sum lrelu(P+b2) = 0.01*(W2.T @ rowsum(Z) + Npair*b2) + accum(relu),
  assembled with tiny per-batch matmuls (identity-matmul moves the
  odd-batch partition halves), then the small MLP on PE/DVE (fp32).

All constants arrive in 4 packed DMA transfers (per-transfer overhead
~0.6us dominates small loads).
"""

import os
import sys

import numpy as np

for _p in ("/opt/trn_rl_repo",):
    if os.path.isdir(_p) and _p not in sys.path:
        sys.path.insert(0, _p)

import operator

import concourse.bass as bass
import concourse.tile as tile
from concourse import bacc, mybir
from concourse.bass import _add_dep_helper

B, C = 32, 128
L = 144
HID = 64
NCORES = 8
BPC = 4  # batches per core
NPAIR = L * L  # 20736
SLOPE = 0.01
LIN_COEF = SLOPE          # weight of the exact linear term
RELU_COEF = 1.0 - SLOPE   # weight of the relu-sum term
PSUM_FD = 2048
SCH = [8, 16, 24, 32, 32, 32]  # i-chunk ramp, capped so consumers never wait
N_PTILES = (NPAIR + PSUM_FD - 1) // PSUM_FD  # 11 psum tiles per group

# fp32 constant pack column map
_C_GA2 = 0          # [128, 144]
_C_GB2 = 144        # [128, 144]
_C_B2C = 288        # [128, 1]
_C_W2S = 289        # [128, 64] (0.01*W2 duplicated into both halves)
_C_I64 = 353        # [128, 64] (identity duplicated into both halves)
_C_WP = 417         # [64, 64]
_C_WO = 481         # [64, 64]
_C_BP4 = 545        # [64, 4]
_C_BO4 = 549        # [64, 4]
_C_C2 = 553         # [64, 1] (0.01 * NPAIR * b2 as a per-partition column)
_C32_COLS = 554

_cache: dict = {}


def _register_op():
    """Register the fused lrelu(Src0 + Src1) custom DVE op at runtime."""
    from concourse import dve_ops
    from concourse.dve_spec import Spec, Src0, Src1, C0, maxx, lower, _has_src1
    from concourse.dve_uop import DveOpSpec

    name = "LRELU_ADD_ANT"
    if name in dve_ops._SUB_OPCODE_FOR_NAME:
        return next(o for o in dve_ops.OPS if o.name == name)

    def _ref(in0, in1, s0, s1, imm2):
        z = np.asarray(in0, np.float32) + np.asarray(in1, np.float32)
        s0v = s0 if isinstance(s0, float) else np.asarray(s0, np.float32)
        out = np.maximum(z, z * s0v)
        acc = out.reshape(out.shape[0], -1).sum(axis=-1, keepdims=True)
        return out, acc.astype(np.float32)

    _z = Src0 + Src1
    spec = Spec(body=maxx(_z, _z * C0), accum=operator.add, reference=_ref)
    op = dve_ops.DveOp(name, spec, subdim=False, uops_sha={})
    dve_ops.OPS.append(op)
    row = dve_ops._CUSTOM_DVE_ROW_BASE + len(dve_ops.OPS) - 1
    assert row < 0x20
    dve_ops._SUB_OPCODE_FOR_NAME[name] = row
    dve_ops.CUSTOM_DVE_SPECS[name] = spec
    for ver in ("v3", "v4"):
        try:
            uops = lower(spec, ver=ver)
            sha = DveOpSpec(
                name=name, opcode=row, uops=uops, rd1_en=_has_src1(spec)
            ).sha(ver)
            op.uops_sha[ver] = sha
        except Exception:
            pass
    return op


def _bcast_in0(ap, S):
    """[128, L] -> [128, S, L] repeating the whole tile S times (0-stride)."""
    return bass.AP(ap.tensor, ap.offset, [ap.ap[0], [0, S], *ap.ap[1:]])


def _bcast_in1(ap, n_inner):
    """[128, S] -> [128, S, n_inner] repeating each column (0-stride inner)."""
    return bass.AP(ap.tensor, ap.offset, [*ap.ap, [0, n_inner]])


def build_nc():
    """Build the Bass module (identical for every core)."""
    LRELU = _register_op()
    nc = bacc.Bacc(trn_type="TRN2")
    f32 = mybir.dt.float32
    f16 = mybir.dt.float16
    bf16 = mybir.dt.bfloat16
    AF = mybir.ActivationFunctionType

    d_xall = nc.dram_tensor("xall", [BPC, 128, L], f16, kind="ExternalInput")
    d_pk16 = nc.dram_tensor("pk16", [128, 2 * HID], f16, kind="ExternalInput")
    d_pkbf = nc.dram_tensor("pkbf", [128, 2 * HID], bf16, kind="ExternalInput")
    d_pk32 = nc.dram_tensor("pk32", [128, _C32_COLS], f32, kind="ExternalInput")
    d_out = nc.dram_tensor("out", [HID, BPC], f32, kind="ExternalOutput")

    with tile.TileContext(nc) as tc:
        with (
            tc.tile_pool(name="const", bufs=1) as cp,
            tc.tile_pool(name="g", bufs=2) as gp,
            tc.tile_pool(name="zl", bufs=4) as zlp,
            tc.tile_pool(name="trash", bufs=2) as trp,
            tc.tile_pool(name="small", bufs=1) as smp,
            tc.tile_pool(name="psum", bufs=2, space=bass.MemorySpace.PSUM) as pp,
        ):
            xall = cp.tile([128, BPC * L], f16, tag="xall")
            # pack 4 batches along the free dim in one transfer
            src = d_xall[:]  # [BPC, 128, L]
            src_perm = bass.AP(
                src.tensor, src.offset, [src.ap[1], src.ap[0], src.ap[2]]
            )
            nc.sync.dma_start(xall[:], src_perm)
            pk16 = cp.tile([128, 2 * HID], f16, tag="pk16")
            nc.sync.dma_start(pk16[:], d_pk16[:])
            pk32 = cp.tile([128, _C32_COLS], f32, tag="pk32")
            nc.sync.dma_start(pk32[:], d_pk32[:])
            pkbf = cp.tile([128, 2 * HID], bf16, tag="pkbf")
            nc.sync.dma_start(pkbf[:], d_pkbf[:])

            t_xf = [xall[:, L * b : L * (b + 1)] for b in range(BPC)]
            t_wa = pk16[:, 0:HID]
            t_wb = pk16[:, HID : 2 * HID]
            t_whi = pkbf[:, 0:HID]
            t_wlo = pkbf[:, HID : 2 * HID]
            t_ga2 = pk32[:, _C_GA2 : _C_GA2 + L]
            t_gb2 = pk32[:, _C_GB2 : _C_GB2 + L]
            t_b2c = pk32[:, _C_B2C : _C_B2C + 1]
            t_w2s = pk32[:, _C_W2S : _C_W2S + HID]
            t_i64 = pk32[:, _C_I64 : _C_I64 + HID]
            t_wp = pk32[0:HID, _C_WP : _C_WP + HID]
            t_wo = pk32[0:HID, _C_WO : _C_WO + HID]
            t_bp4 = pk32[0:HID, _C_BP4 : _C_BP4 + BPC]
            t_bo4 = pk32[0:HID, _C_BO4 : _C_BO4 + BPC]
            t_c2 = pk32[0:HID, _C_C2 : _C_C2 + 1]

            accz = smp.tile([128, 16], f32, tag="accz")
            absc = smp.tile([128, 32], f32, tag="absc")
            zsumg = smp.tile([128, 2], f32, tag="zsumg")
            asumg = smp.tile([128, 2], f32, tag="asumg")

            # ---- prep: gi2 / gjb2 for both groups (PSUM halves per batch) --
            gi2s, gjb2s = [], []
            for g in range(2):
                ps_gi = pp.tile([128, L], f32, tag="mm")
                nc.tensor.matmul(ps_gi[0:64, :], t_wa, t_xf[2 * g])
                nc.tensor.matmul(ps_gi[64:128, :], t_wa, t_xf[2 * g + 1])
                gi2 = gp.tile([128, L], f32, tag="gi2")
                nc.vector.tensor_add(gi2[:], ps_gi[:], t_ga2)
                ps_gj = pp.tile([128, L], f32, tag="mm")
                nc.tensor.matmul(ps_gj[0:64, :], t_wb, t_xf[2 * g])
                nc.tensor.matmul(ps_gj[64:128, :], t_wb, t_xf[2 * g + 1])
                gjb2 = gp.tile([128, L], f32, tag="gjb2")
                nc.vector.tensor_add(gjb2[:], ps_gj[:], t_gb2)
                gi2s.append(gi2)
                gjb2s.append(gjb2)

            # ---- main: per group, fused-lrelu Z tiles -> matmuls -> ACT ----
            for g in range(2):
                gi2, gjb2 = gi2s[g], gjb2s[g]
                segs = []  # (tile, start_col, n_cols)
                i0 = 0
                zi_insts = []
                for ci, S in enumerate(SCH):
                    zt = zlp.tile([128, S * L], bf16, tag="zl")
                    in0 = _bcast_in0(gi2[:, 0:L], S)
                    in1 = _bcast_in1(gjb2[:, i0 : i0 + S], L)
                    zi = nc.vector._custom_dve(
                        LRELU,
                        out=zt[:],
                        in0=in0,
                        in1=in1,
                        s0=SLOPE,
                        accum_out=accz[:, 8 * g + ci : 8 * g + ci + 1],
                    )
                    zi_insts.append(zi)
                    segs.append((zt, i0 * L, S * L))
                    i0 += S

                def seg_for(c):
                    for (zt, s0_, n_) in segs:
                        if s0_ <= c < s0_ + n_:
                            return zt, c - s0_, s0_ + n_ - c
                    raise AssertionError(c)

                c = 0
                ti = 0
                ps = None
                act_insts = []
                while c < NPAIR:
                    pcol = c % PSUM_FD
                    if pcol == 0:
                        ps = pp.tile([128, PSUM_FD], f32, tag="mm")
                    zt, zoff, zleft = seg_for(c)
                    n = min(512 - (pcol % 512), zleft, NPAIR - c)
                    for h in range(2):  # batch half: partitions 64h..64h+63
                        r = slice(64 * h, 64 * h + 64)
                        nc.tensor.matmul(
                            ps[r, pcol : pcol + n],
                            t_whi[r, :],
                            zt[r, zoff : zoff + n],
                            start=True,
                            stop=False,
                        )
                        nc.tensor.matmul(
                            ps[r, pcol : pcol + n],
                            t_wlo[r, :],
                            zt[r, zoff : zoff + n],
                            start=False,
                            stop=True,
                        )
                    c += n
                    if c % PSUM_FD == 0 or c == NPAIR:
                        fd = pcol + n
                        tr = trp.tile([128, PSUM_FD], f16, tag="tr")
                        ai = nc.scalar.activation(
                            tr[:, 0:fd],
                            ps[:, 0:fd],
                            AF.Relu,
                            bias=t_b2c,
                            scale=RELU_COEF,
                            accum_out=absc[:, 16 * g + ti : 16 * g + ti + 1],
                        )
                        act_insts.append(ai)
                        ti += 1
                assert ti == N_PTILES

                # per-group: fold the per-chunk accumulators. The reduces
                # must wait for the accum_out (second-output) writes, which
                # Tile's dependency tracker does not see — add explicit edges.
                rz = nc.vector.tensor_reduce(
                    zsumg[:, g : g + 1],
                    accz[:, 8 * g : 8 * g + len(SCH)],
                    axis=mybir.AxisListType.X,
                    op=mybir.AluOpType.add,
                )
                for zi in zi_insts:
                    _add_dep_helper(rz.ins, zi.ins, sync=True, reason="accz accum_out")
                ra = nc.vector.tensor_reduce(
                    asumg[:, g : g + 1],
                    absc[:, 16 * g : 16 * g + N_PTILES],
                    axis=mybir.AxisListType.X,
                    op=mybir.AluOpType.add,
                )
                for ai in act_insts:
                    _add_dep_helper(ra.ins, ai.ins, sync=True, reason="absc accum_out")

            # ---- tail: s = 0.01*(W2.T zsum + N b2) + relu-accum, tiny MLP --
            zsum_all = smp.tile([HID, BPC], f32, tag="zsum_all")
            asum_all = smp.tile([HID, BPC], f32, tag="asum_all")
            for b in range(BPC):
                g, h = divmod(b, 2)
                r = slice(64 * h, 64 * h + 64)
                nc.sync.dma_start(zsum_all[0:64, b : b + 1], zsumg[r, g : g + 1])
                nc.sync.dma_start(asum_all[0:64, b : b + 1], asumg[r, g : g + 1])
            lz = pp.tile([HID, BPC], f32, tag="mm")
            nc.tensor.matmul(lz[:], t_w2s[0:HID, :], zsum_all[:])
            s_all = smp.tile([HID, BPC], f32, tag="s_all")
            nc.vector.tensor_scalar_add(s_all[:], lz[:], t_c2)
            nc.vector.tensor_add(s_all[:], s_all[:], asum_all[:])

            p1 = pp.tile([HID, BPC], f32, tag="mm")
            nc.tensor.matmul(p1[:], t_wp, s_all[:])
            h1 = smp.tile([HID, BPC], f32, tag="h1")
            nc.vector._custom_dve(LRELU, out=h1[:], in0=p1[:], in1=t_bp4, s0=SLOPE)
            p2 = pp.tile([HID, BPC], f32, tag="mm")
            nc.tensor.matmul(p2[:], t_wo, h1[:])
            fin = smp.tile([HID, BPC], f32, tag="fin")
            nc.vector._custom_dve(LRELU, out=fin[:], in0=p2[:], in1=t_bo4, s0=SLOPE)
            nc.sync.dma_start(d_out[:], fin[:])

    nc.compile()
    return nc


def host_prep(inputs):
    """Host-side weight preprocessing -> shared input map + per-core xall."""
    x_img = np.asarray(inputs["x_img"], np.float32)
    W1 = np.asarray(inputs["W1"], np.float32)
    b1 = np.asarray(inputs["b1"], np.float32)
    W2 = np.asarray(inputs["W2"], np.float32)
    b2 = np.asarray(inputs["b2"], np.float32)
    Wp = np.asarray(inputs["Wp"], np.float32)
    bp = np.asarray(inputs["bp"], np.float32)
    Wo = np.asarray(inputs["Wo"], np.float32)
    bo = np.asarray(inputs["bo"], np.float32)
    import ml_dtypes

    BF = ml_dtypes.bfloat16

    x = x_img.reshape(B, C, L)  # [b, c, l]
    coords = np.arange(L, dtype=np.float32)
    GaT = (coords[:, None] * W1[C][None, :]).T  # [64, 144]
    GbT = (coords[:, None] * W1[C + 1 + C][None, :] + b1[None, :]).T
    W2hi = W2.astype(BF)
    W2lo = (W2 - W2hi.astype(np.float32)).astype(BF)

    pk16 = np.zeros((128, 2 * HID), np.float16)
    pk16[:, 0:HID] = W1[:C].astype(np.float16)
    pk16[:, HID : 2 * HID] = W1[C + 1 : C + 1 + C].astype(np.float16)

    pkbf = np.zeros((128, 2 * HID), BF)
    pkbf[0:64, 0:HID] = W2hi
    pkbf[64:128, 0:HID] = W2hi
    pkbf[0:64, HID:] = W2lo
    pkbf[64:128, HID:] = W2lo

    pk32 = np.zeros((128, _C32_COLS), np.float32)
    pk32[:, _C_GA2 : _C_GA2 + L] = np.concatenate([GaT, GaT], 0)
    pk32[:, _C_GB2 : _C_GB2 + L] = np.concatenate([GbT, GbT], 0)
    pk32[:, _C_B2C] = np.tile(RELU_COEF * b2, 2)
    pk32[0:64, _C_W2S : _C_W2S + HID] = LIN_COEF * W2
    pk32[64:128, _C_W2S : _C_W2S + HID] = LIN_COEF * W2
    eye = np.eye(HID, dtype=np.float32)
    pk32[0:64, _C_I64 : _C_I64 + HID] = eye
    pk32[64:128, _C_I64 : _C_I64 + HID] = eye
    pk32[0:HID, _C_WP : _C_WP + HID] = Wp
    pk32[0:HID, _C_WO : _C_WO + HID] = Wo
    pk32[0:HID, _C_BP4 : _C_BP4 + BPC] = np.repeat(bp[:, None], BPC, axis=1)
    pk32[0:HID, _C_BO4 : _C_BO4 + BPC] = np.repeat(bo[:, None], BPC, axis=1)
    pk32[0:HID, _C_C2] = LIN_COEF * NPAIR * b2

    base = {
        "pk16": np.ascontiguousarray(pk16),
        "pkbf": np.ascontiguousarray(pkbf),
        "pk32": np.ascontiguousarray(pk32),
    }
    in_maps = []
    for k in range(NCORES):
        m = dict(base)
        m["xall"] = np.ascontiguousarray(
            x[BPC * k : BPC * (k + 1)].astype(np.float16)
        )
        in_maps.append(m)
    return in_maps


def kernel(**inputs) -> np.ndarray:
    from concourse.bass_utils import run_bass_kernel_spmd

    if "nc" not in _cache:
        _cache["nc"] = build_nc()
    nc = _cache["nc"]
    in_maps = host_prep(inputs)
    res = run_bass_kernel_spmd(nc, in_maps, core_ids=list(range(NCORES)))
    out = np.concatenate([r["out"].T for r in res.results], axis=0)  # [32, 64]
    return np.ascontiguousarray(out, np.float32)


# revision 20
# speedup vs baseline: 1.2668x; 1.0447x over previous
"""Trainium2 Bass kernel for nn_BatchRelationalModule (gnn_message_passing).

Reference computation (per batch b of 32):
  x = [imgfeat(128) | coord] per position l in 0..143            # [L, 129]
  gi = x @ W1[:129]   (indexed by j);  gjb = x @ W1[129:] + b1   # [L, 64]
  A[:, (i,j)] = lrelu(gi[j] + gjb[i])                            # [64, L*L]
  P = W2.T @ A + b2;  s = sum_{i,j} lrelu(P)                     # [64]
  out = lrelu(lrelu(s @ Wp + bp) @ Wo + bo)                      # [64]

Sharding: data-parallel over batch, 4 batches per core (2 groups of 2
batches stacked on SBUF partitions: rows 0-63 = even batch features,
rows 64-127 = odd batch features).

Per-core device pipeline (per 2-batch group):
  PE  : gi/gjb prep matmuls (x @ W, fp16) into PSUM halves
  DVE : custom fused op  Z = lrelu(gi_bcast + gjb_bcast)  (one pass,
        broadcast via 0-stride access patterns, fp32 in / fp16 out)
        + accum_out = rowsum(Z) in fp32
  PE  : W2.T @ Z as bf16 hi+lo accumulating matmul pairs (~16-bit
        effective weight mantissa; bf16 keeps the fp32 exponent range so
        the lo part never denormal-flushes), col-tiled so both batches
        run concurrently and PSUM packs [128, fd]
  ACT : relu(0.99*(P + b2)) with per-partition bias + fused accum_out
  final: sum lrelu(P+b2) = 0.01*(W2.T @ rowsum(Z) + Npair*b2) + accum(relu),
  assembled with tiny per-batch matmuls (identity-matmul moves the
  odd-batch partition halves), then the small MLP on PE/DVE (fp32).

All constants arrive in 4 packed DMA transfers (per-transfer overhead
~0.6us dominates small loads).
"""

import os
import sys

import numpy as np

for _p in ("/opt/trn_rl_repo",):
    if os.path.isdir(_p) and _p not in sys.path:
        sys.path.insert(0, _p)

import operator

import concourse.bass as bass
import concourse.tile as tile
from concourse import bacc, mybir
from concourse.bass import _add_dep_helper

B, C = 32, 128
L = 144
HID = 64
NCORES = 8
BPC = 4  # batches per core
NPAIR = L * L  # 20736
SLOPE = 0.01
LIN_COEF = SLOPE          # weight of the exact linear term
RELU_COEF = 1.0 - SLOPE   # weight of the relu-sum term
PSUM_FD = 2048
SCH = [8, 16, 24, 32, 32, 32]  # i-chunk ramp, capped so consumers never wait
# Per-group ACT tile plans. Group 0 starts with small tiles so the first
# ACT fires as soon as the first Z chunk lands; group 1 is already
# pipelined and uses full tiles.
PLANS = [[512, 1024] + [2048] * 9 + [768], [2048] * 10 + [256]]
assert all(sum(p) == NPAIR for p in PLANS)

# fp32 constant pack column map
_C_GA2 = 0          # [128, 144]
_C_GB2 = 144        # [128, 144]
_C_B2C = 288        # [128, 1]
_C_W2S = 289        # [128, 64] (0.01*W2 duplicated into both halves)
_C_I64 = 353        # [128, 64] (identity duplicated into both halves)
_C_WP = 417         # [64, 64]
_C_WO = 481         # [64, 64]
_C_BP4 = 545        # [64, 4]
_C_BO4 = 549        # [64, 4]
_C_C2 = 553         # [64, 1] (0.01 * NPAIR * b2 as a per-partition column)
_C32_COLS = 554

_cache: dict = {}


def _register_op():
    """Register the fused lrelu(Src0 + Src1) custom DVE op at runtime."""
    from concourse import dve_ops
    from concourse.dve_spec import Spec, Src0, Src1, C0, maxx, lower, _has_src1
    from concourse.dve_uop import DveOpSpec

    name = "LRELU_ADD_ANT"
    if name in dve_ops._SUB_OPCODE_FOR_NAME:
        return next(o for o in dve_ops.OPS if o.name == name)

    def _ref(in0, in1, s0, s1, imm2):
        z = np.asarray(in0, np.float32) + np.asarray(in1, np.float32)
        s0v = s0 if isinstance(s0, float) else np.asarray(s0, np.float32)
        out = np.maximum(z, z * s0v)
        acc = out.reshape(out.shape[0], -1).sum(axis=-1, keepdims=True)
        return out, acc.astype(np.float32)

    _z = Src0 + Src1
    spec = Spec(body=maxx(_z, _z * C0), accum=operator.add, reference=_ref)
    op = dve_ops.DveOp(name, spec, subdim=False, uops_sha={})
    dve_ops.OPS.append(op)
    row = dve_ops._CUSTOM_DVE_ROW_BASE + len(dve_ops.OPS) - 1
    assert row < 0x20
    dve_ops._SUB_OPCODE_FOR_NAME[name] = row
    dve_ops.CUSTOM_DVE_SPECS[name] = spec
    for ver in ("v3", "v4"):
        try:
            uops = lower(spec, ver=ver)
            sha = DveOpSpec(
                name=name, opcode=row, uops=uops, rd1_en=_has_src1(spec)
            ).sha(ver)
            op.uops_sha[ver] = sha
        except Exception:
            pass
    return op


def _bcast_in0(ap, S):
    """[128, L] -> [128, S, L] repeating the whole tile S times (0-stride)."""
    return bass.AP(ap.tensor, ap.offset, [ap.ap[0], [0, S], *ap.ap[1:]])


def _bcast_in1(ap, n_inner):
    """[128, S] -> [128, S, n_inner] repeating each column (0-stride inner)."""
    return bass.AP(ap.tensor, ap.offset, [*ap.ap, [0, n_inner]])


def build_nc():
    """Build the Bass module (identical for every core)."""
    LRELU = _register_op()
    nc = bacc.Bacc(trn_type="TRN2")
    f32 = mybir.dt.float32
    f16 = mybir.dt.float16
    bf16 = mybir.dt.bfloat16
    AF = mybir.ActivationFunctionType

    d_xall = nc.dram_tensor("xall", [BPC, 128, L], f16, kind="ExternalInput")
    d_pk16 = nc.dram_tensor("pk16", [128, 2 * HID], f16, kind="ExternalInput")
    d_pkbf = nc.dram_tensor("pkbf", [128, 2 * HID], bf16, kind="ExternalInput")
    d_pk32 = nc.dram_tensor("pk32", [128, _C32_COLS], f32, kind="ExternalInput")
    d_out = nc.dram_tensor("out", [HID, BPC], f32, kind="ExternalOutput")

    with tile.TileContext(nc) as tc:
        with (
            tc.tile_pool(name="const", bufs=1) as cp,
            tc.tile_pool(name="g", bufs=2) as gp,
            tc.tile_pool(name="zl", bufs=4) as zlp,
            tc.tile_pool(name="trash", bufs=2) as trp,
            tc.tile_pool(name="small", bufs=1) as smp,
            tc.tile_pool(name="psum", bufs=2, space=bass.MemorySpace.PSUM) as pp,
        ):
            xall = cp.tile([128, BPC * L], f16, tag="xall")
            # pack 4 batches along the free dim in one transfer
            src = d_xall[:]  # [BPC, 128, L]
            src_perm = bass.AP(
                src.tensor, src.offset, [src.ap[1], src.ap[0], src.ap[2]]
            )
            nc.sync.dma_start(xall[:], src_perm)
            pk16 = cp.tile([128, 2 * HID], f16, tag="pk16")
            nc.sync.dma_start(pk16[:], d_pk16[:])
            pk32 = cp.tile([128, _C32_COLS], f32, tag="pk32")
            nc.sync.dma_start(pk32[:], d_pk32[:])
            pkbf = cp.tile([128, 2 * HID], bf16, tag="pkbf")
            nc.sync.dma_start(pkbf[:], d_pkbf[:])

            t_xf = [xall[:, L * b : L * (b + 1)] for b in range(BPC)]
            t_wa = pk16[:, 0:HID]
            t_wb = pk16[:, HID : 2 * HID]
            t_whi = pkbf[:, 0:HID]
            t_wlo = pkbf[:, HID : 2 * HID]
            t_ga2 = pk32[:, _C_GA2 : _C_GA2 + L]
            t_gb2 = pk32[:, _C_GB2 : _C_GB2 + L]
            t_b2c = pk32[:, _C_B2C : _C_B2C + 1]
            t_w2s = pk32[:, _C_W2S : _C_W2S + HID]
            t_i64 = pk32[:, _C_I64 : _C_I64 + HID]
            t_wp = pk32[0:HID, _C_WP : _C_WP + HID]
            t_wo = pk32[0:HID, _C_WO : _C_WO + HID]
            t_bp4 = pk32[0:HID, _C_BP4 : _C_BP4 + BPC]
            t_bo4 = pk32[0:HID, _C_BO4 : _C_BO4 + BPC]
            t_c2 = pk32[0:HID, _C_C2 : _C_C2 + 1]

            accz = smp.tile([128, 16], f32, tag="accz")
            absc = smp.tile([128, 32], f32, tag="absc")
            zsumg = smp.tile([128, 2], f32, tag="zsumg")
            asumg = smp.tile([128, 2], f32, tag="asumg")

            # ---- prep: gi2 / gjb2 for both groups (PSUM halves per batch) --
            gi2s, gjb2s = [], []
            for g in range(2):
                ps_gi = pp.tile([128, L], f32, tag="mm")
                nc.tensor.matmul(ps_gi[0:64, :], t_wa, t_xf[2 * g])
                nc.tensor.matmul(ps_gi[64:128, :], t_wa, t_xf[2 * g + 1])
                gi2 = gp.tile([128, L], f32, tag="gi2")
                nc.vector.tensor_add(gi2[:], ps_gi[:], t_ga2)
                ps_gj = pp.tile([128, L], f32, tag="mm")
                nc.tensor.matmul(ps_gj[0:64, :], t_wb, t_xf[2 * g])
                nc.tensor.matmul(ps_gj[64:128, :], t_wb, t_xf[2 * g + 1])
                gjb2 = gp.tile([128, L], f32, tag="gjb2")
                nc.vector.tensor_add(gjb2[:], ps_gj[:], t_gb2)
                gi2s.append(gi2)
                gjb2s.append(gjb2)

            # ---- main: per group, fused-lrelu Z tiles -> matmuls -> ACT ----
            for g in range(2):
                gi2, gjb2 = gi2s[g], gjb2s[g]
                segs = []  # (tile, start_col, n_cols)
                i0 = 0
                zi_insts = []
                for ci, S in enumerate(SCH):
                    zt = zlp.tile([128, S * L], bf16, tag="zl")
                    in0 = _bcast_in0(gi2[:, 0:L], S)
                    in1 = _bcast_in1(gjb2[:, i0 : i0 + S], L)
                    zi = nc.vector._custom_dve(
                        LRELU,
                        out=zt[:],
                        in0=in0,
                        in1=in1,
                        s0=SLOPE,
                        accum_out=accz[:, 8 * g + ci : 8 * g + ci + 1],
                    )
                    zi_insts.append(zi)
                    segs.append((zt, i0 * L, S * L))
                    i0 += S

                def seg_for(c):
                    for (zt, s0_, n_) in segs:
                        if s0_ <= c < s0_ + n_:
                            return zt, c - s0_, s0_ + n_ - c
                    raise AssertionError(c)

                c = 0
                act_insts = []
                for ti, fd in enumerate(PLANS[g]):
                    ps = pp.tile([128, PSUM_FD], f32, tag="mm")
                    pcol = 0
                    while pcol < fd:
                        zt, zoff, zleft = seg_for(c)
                        n = min(512 - (pcol % 512), zleft, fd - pcol)
                        for h in range(2):  # batch half: partitions 64h..
                            r = slice(64 * h, 64 * h + 64)
                            nc.tensor.matmul(
                                ps[r, pcol : pcol + n],
                                t_whi[r, :],
                                zt[r, zoff : zoff + n],
                                start=True,
                                stop=False,
                            )
                            nc.tensor.matmul(
                                ps[r, pcol : pcol + n],
                                t_wlo[r, :],
                                zt[r, zoff : zoff + n],
                                start=False,
                                stop=True,
                            )
                        c += n
                        pcol += n
                    tr = trp.tile([128, PSUM_FD], f16, tag="tr")
                    ai = nc.scalar.activation(
                        tr[:, 0:fd],
                        ps[:, 0:fd],
                        AF.Relu,
                        bias=t_b2c,
                        scale=RELU_COEF,
                        accum_out=absc[:, 16 * g + ti : 16 * g + ti + 1],
                    )
                    act_insts.append(ai)
                assert c == NPAIR

                # per-group: fold the per-chunk accumulators. The reduces
                # must wait for the accum_out (second-output) writes, which
                # Tile's dependency tracker does not see — add explicit edges.
                rz = nc.vector.tensor_reduce(
                    zsumg[:, g : g + 1],
                    accz[:, 8 * g : 8 * g + len(SCH)],
                    axis=mybir.AxisListType.X,
                    op=mybir.AluOpType.add,
                )
                for zi in zi_insts:
                    _add_dep_helper(rz.ins, zi.ins, sync=True, reason="accz accum_out")
                ra = nc.vector.tensor_reduce(
                    asumg[:, g : g + 1],
                    absc[:, 16 * g : 16 * g + len(PLANS[g])],
                    axis=mybir.AxisListType.X,
                    op=mybir.AluOpType.add,
                )
                for ai in act_insts:
                    _add_dep_helper(ra.ins, ai.ins, sync=True, reason="absc accum_out")

            # ---- tail: s = 0.01*(W2.T zsum + N b2) + relu-accum, tiny MLP --
            zsum_all = smp.tile([HID, BPC], f32, tag="zsum_all")
            asum_all = smp.tile([HID, BPC], f32, tag="asum_all")
            for b in range(BPC):
                g, h = divmod(b, 2)
                r = slice(64 * h, 64 * h + 64)
                if h == 0:
                    nc.vector.tensor_copy(
                        zsum_all[0:64, b : b + 1], zsumg[r, g : g + 1]
                    )
                    nc.vector.tensor_copy(
                        asum_all[0:64, b : b + 1], asumg[r, g : g + 1]
                    )
                else:
                    nc.sync.dma_start(zsum_all[0:64, b : b + 1], zsumg[r, g : g + 1])
                    nc.sync.dma_start(asum_all[0:64, b : b + 1], asumg[r, g : g + 1])
            lz = pp.tile([HID, BPC], f32, tag="mm")
            nc.tensor.matmul(lz[:], t_w2s[0:HID, :], zsum_all[:])
            s_all = smp.tile([HID, BPC], f32, tag="s_all")
            nc.vector.tensor_scalar_add(s_all[:], lz[:], t_c2)
            nc.vector.tensor_add(s_all[:], s_all[:], asum_all[:])

            p1 = pp.tile([HID, BPC], f32, tag="mm")
            nc.tensor.matmul(p1[:], t_wp, s_all[:])
            h1 = smp.tile([HID, BPC], f32, tag="h1")
            nc.vector._custom_dve(LRELU, out=h1[:], in0=p1[:], in1=t_bp4, s0=SLOPE)
            p2 = pp.tile([HID, BPC], f32, tag="mm")
            nc.tensor.matmul(p2[:], t_wo, h1[:])
            fin = smp.tile([HID, BPC], f32, tag="fin")
            nc.vector._custom_dve(LRELU, out=fin[:], in0=p2[:], in1=t_bo4, s0=SLOPE)
            nc.sync.dma_start(d_out[:], fin[:])

    nc.compile()
    return nc


def host_prep(inputs):
    """Host-side weight preprocessing -> shared input map + per-core xall."""
    x_img = np.asarray(inputs["x_img"], np.float32)
    W1 = np.asarray(inputs["W1"], np.float32)
    b1 = np.asarray(inputs["b1"], np.float32)
    W2 = np.asarray(inputs["W2"], np.float32)
    b2 = np.asarray(inputs["b2"], np.float32)
    Wp = np.asarray(inputs["Wp"], np.float32)
    bp = np.asarray(inputs["bp"], np.float32)
    Wo = np.asarray(inputs["Wo"], np.float32)
    bo = np.asarray(inputs["bo"], np.float32)
    import ml_dtypes

    BF = ml_dtypes.bfloat16

    x = x_img.reshape(B, C, L)  # [b, c, l]
    coords = np.arange(L, dtype=np.float32)
    GaT = (coords[:, None] * W1[C][None, :]).T  # [64, 144]
    GbT = (coords[:, None] * W1[C + 1 + C][None, :] + b1[None, :]).T
    W2hi = W2.astype(BF)
    W2lo = (W2 - W2hi.astype(np.float32)).astype(BF)

    pk16 = np.zeros((128, 2 * HID), np.float16)
    pk16[:, 0:HID] = W1[:C].astype(np.float16)
    pk16[:, HID : 2 * HID] = W1[C + 1 : C + 1 + C].astype(np.float16)

    pkbf = np.zeros((128, 2 * HID), BF)
    pkbf[0:64, 0:HID] = W2hi
    pkbf[64:128, 0:HID] = W2hi
    pkbf[0:64, HID:] = W2lo
    pkbf[64:128, HID:] = W2lo

    pk32 = np.zeros((128, _C32_COLS), np.float32)
    pk32[:, _C_GA2 : _C_GA2 + L] = np.concatenate([GaT, GaT], 0)
    pk32[:, _C_GB2 : _C_GB2 + L] = np.concatenate([GbT, GbT], 0)
    pk32[:, _C_B2C] = np.tile(RELU_COEF * b2, 2)
    pk32[0:64, _C_W2S : _C_W2S + HID] = LIN_COEF * W2
    pk32[64:128, _C_W2S : _C_W2S + HID] = LIN_COEF * W2
    eye = np.eye(HID, dtype=np.float32)
    pk32[0:64, _C_I64 : _C_I64 + HID] = eye
    pk32[64:128, _C_I64 : _C_I64 + HID] = eye
    pk32[0:HID, _C_WP : _C_WP + HID] = Wp
    pk32[0:HID, _C_WO : _C_WO + HID] = Wo
    pk32[0:HID, _C_BP4 : _C_BP4 + BPC] = np.repeat(bp[:, None], BPC, axis=1)
    pk32[0:HID, _C_BO4 : _C_BO4 + BPC] = np.repeat(bo[:, None], BPC, axis=1)
    pk32[0:HID, _C_C2] = LIN_COEF * NPAIR * b2

    base = {
        "pk16": np.ascontiguousarray(pk16),
        "pkbf": np.ascontiguousarray(pkbf),
        "pk32": np.ascontiguousarray(pk32),
    }
    in_maps = []
    for k in range(NCORES):
        m = dict(base)
        m["xall"] = np.ascontiguousarray(
            x[BPC * k : BPC * (k + 1)].astype(np.float16)
        )
        in_maps.append(m)
    return in_maps


def kernel(**inputs) -> np.ndarray:
    from concourse.bass_utils import run_bass_kernel_spmd

    if "nc" not in _cache:
        _cache["nc"] = build_nc()
    nc = _cache["nc"]
    in_maps = host_prep(inputs)
    res = run_bass_kernel_spmd(nc, in_maps, core_ids=list(range(NCORES)))
    out = np.concatenate([r["out"].T for r in res.results], axis=0)  # [32, 64]
    return np.ascontiguousarray(out, np.float32)
